# revision 47
# baseline (speedup 1.0000x reference)
"""Nonstationary Matern-5/2 kernel matrix on 8 Trainium2 NeuronCores.

Math: out[i,j] = (1 + u + u^2/3) * exp(-u),  u = sqrt5 * r_ij * (s(x_i)+s(y_j))
where r_ij = ||x_i - y_j|| and s() is a tiny MLP (Linear-selu-Linear-softplus).

Key trick: u^2 = 5*r2*S^2 where r2 (rank-5 in outer-product terms) and
S^2 = (sx+sy)^2 (rank-3) multiply elementwise into a rank-15 sum, so ONE
matmul per output tile produces w = u^2 (+ a constant clamp column).
The contraction runs in fp16 (1 PE cycle/row vs 4 for fp32) with each
fp32 column value split hi/lo (hi = f16(a), lo = f16(a - hi)) across a
K=46 contraction [Xh;Xh;Xl] x [Yh;Yl;Yh] + clamp row, recovering the
fp32 product to ~2^-22 relative. Then per tile: u = sqrt(w) on ACT,
e = exp(-u - ln3) on ACT (phased so sqrt/exp activation tables do not
thrash), and out = ((u+1.5)^2 + 0.75) * e in one fused custom DVE op
(since 1+u+u^2/3 = ((u+1.5)^2 + 0.75)/3), written as fp16 and widened
to fp32 on the host (k in [0,1], so f16 costs <5e-4 abs).

The scale MLP runs on-device in a 2-half blockdiag layout ([128, 4608]
for the 9216 points), with selu fused into one custom DVE select op and
softplus as exp+ln (same activation table set). Transposed copies of
x/y are passed from the host so the [3, N] point layout loads as
contiguous rows instead of a 4-byte-strided gather.

Sharding: data-parallel over rows of x; each core computes a [1024, 8192]
block; y and MLP params replicated. The per-point scale MLP runs on-device
on every core (x-shard + full y = 9216 points).
"""

import numpy as np

import concourse.bacc as bacc
import concourse.bass as bass
import concourse.mybir as mybir
from concourse.mybir import AluOpType as alu
from concourse.tile import TileContext
from concourse.bass_utils import run_bass_kernel_spmd

import concourse.dve_ops as dve_ops
from concourse.dve_spec import (
    Spec, Src0, Src1, C0, C1, Zero, One, sq, select, lower,
)
from concourse.dve_uop import DveOpSpec

N, M, D, L = 8192, 8192, 3, 64
N_CORES = 8
ROWS = N // N_CORES          # 1024 x-rows per core
N_STRIPS = ROWS // 128       # 8 strips of 128 partitions
GRP = 2048                   # supertile free width
N_GRP = M // GRP             # 4 col groups per strip
MMW = 512                    # fp32 matmul moving width
PHASE_STRIPS = 4             # strips per sqrt/exp table phase

LN3 = float(np.log(3.0))
SELU_L = 1.0507009873554805
SELU_A = 1.6732632423543772
CLAMP_EPS = 1e-2             # w = u^2 clamp floor (vs fp16-split matmul noise)
KSPL = 46                    # contraction: 15 XhYh + 15 XhYl + 15 XlYh + clamp

F32 = mybir.dt.float32
F16 = mybir.dt.float16
F32R = mybir.dt.float32r
Act = mybir.ActivationFunctionType


def _register_dve(name, spec):
    for o in dve_ops.OPS:
        if o.name == name:
            return o
    shas = {}
    for ver in ("v3", "v4"):
        uops = lower(spec, ver=ver)
        shas[ver] = DveOpSpec(name=name, opcode=1, uops=uops, rd1_en=True).sha(ver)
    op = dve_ops.DveOp(name, spec, subdim=False, uops_sha=shas)
    dve_ops.OPS.append(op)
    dve_ops.CUSTOM_DVE_SPECS[name] = spec
    dve_ops._SUB_OPCODE_FOR_NAME[name] = (
        dve_ops._CUSTOM_DVE_ROW_BASE + len(dve_ops.OPS) - 1
    )
    return op


def _register_matern_tail():
    """out = ((in0 + s0)^2 + s1) * in1, one fused DVE instruction."""
    return _register_dve("MATERN_TAIL_ANT", Spec(
        body=(sq(Src0 + C0) + C1) * Src1,
        reference=lambda in0, in1, s0, s1, imm2: (
            ((in0.astype(np.float32) + s0) ** 2 + s1) * in1
        ).astype(np.float32),
    ))


def _register_selu_neg():
    """hsel = -selu(h')/lambda with h' = in0 + s0, in1 = exp(h'):
    select(h' < 0, alpha*(1 - e), -h'), alpha = s1."""

    def ref(in0, in1, s0, s1, imm2):
        hp = in0.astype(np.float32) + s0
        e = in1.astype(np.float32)
        return np.where(hp < 0, s1 * (1.0 - e), -hp).astype(np.float32)

    return _register_dve("SELU_NEG_ANT", Spec(
        body=select((Src0 + C0) < Zero, C1 * (One - Src1), Zero - (Src0 + C0)),
        reference=ref,
    ))


def _register_const(nc, val, dtype=F32):
    key = (dtype, float(val))
    if key in nc.const_aps.aps:
        return
    t = nc.alloc_sbuf_tensor(f"const-{dtype.name}-{val}", [128, 1], dtype)
    nc.gpsimd.memset(t.ap(), float(val))
    nc.const_aps.aps[key] = t.ap()


def build(repeat=1, repeat_a=1):
    tail_op = _register_matern_tail()
    selu_op = _register_selu_neg()
    nc = bacc.Bacc(num_devices=1, debug=False)
    _register_const(nc, -LN3)
    _register_const(nc, 1.0)
    _register_const(nc, 1.5)
    nc.all_engine_barrier()

    x = nc.dram_tensor("x", [ROWS, D], F32, kind="ExternalInput")
    y = nc.dram_tensor("y", [M, D], F32, kind="ExternalInput")
    xTd = nc.dram_tensor("xT", [D, ROWS], F32, kind="ExternalInput")
    yTd = nc.dram_tensor("yT", [D, M], F32, kind="ExternalInput")
    W1 = nc.dram_tensor("W1", [L, D], F32, kind="ExternalInput")
    b1 = nc.dram_tensor("b1", [L], F32, kind="ExternalInput")
    W2 = nc.dram_tensor("W2", [1, L], F32, kind="ExternalInput")
    b2 = nc.dram_tensor("b2", [1], F32, kind="ExternalInput")
    out = nc.dram_tensor("out", [ROWS, M], F16, kind="ExternalOutput")

    NPTS = ROWS + M           # 9216 points: x-shard then y
    HALF = NPTS // 2          # 4608; A = x + y[:3584], B = y[3584:]
    YA = HALF - ROWS          # 3584 y points in half A

    with TileContext(nc) as tc:
        # persistent matmul-column tensors, live for the whole kernel
        with tc.tile_pool(name="keep", bufs=1) as kp:
            ycols = kp.tile([KSPL, M], F16)
            xcols = kp.tile([KSPL, ROWS], F16)
            for _ in range(repeat_a):
                _build_columns(nc, tc, x, y, xTd, yTd, W1, b1, W2, b2,
                               ycols, xcols, selu_op)
            for _ in range(repeat):
                _main_loop(nc, tc, out, ycols, xcols, tail_op)
    nc.compile()
    return nc


def _build_columns(nc, tc, x, y, xTd, yTd, W1, b1, W2, b2, ycols, xcols,
                   selu_op):
    NPTS = ROWS + M           # 9216 points: x-shard then y
    HALF = NPTS // 2          # 4608; A = x + y[:3584], B = y[3584:]
    YA = HALF - ROWS          # 3584 y points in half A

    if True:
        # ---------------- stage A: per-point scales + matmul columns -------
        with tc.tile_pool(name="mlp", bufs=1) as mp, \
             tc.tile_pool(name="mlp_tmp", bufs=2) as mt, \
             tc.tile_pool(name="mlp_psum", bufs=1, space="PSUM") as mpp:
            # points, 2-half blockdiag layout: partitions 0-2 coords of half A,
            # partitions 3-5 coords of half B
            pts6h = mp.tile([2 * D, 1024], F32)
            pts6 = mp.tile([2 * D, HALF], F32)
            nc.sync.dma_start(pts6h[D:2 * D, :].bitcast(F32R),
                              yTd[:, YA:YA + 1024].bitcast(F32R))
            nc.gpsimd.dma_start(pts6h[0:D, :].bitcast(F32R),
                                xTd[:, 0:1024].bitcast(F32R))

            w1t6 = mp.tile([2 * D, 128], F32)
            nc.vector.memset(w1t6[:, :], 0.0)
            w1T = W1[:, :].rearrange("l d -> d l")
            nc.sync.dma_start(w1t6[0:D, 0:L].bitcast(F32R), w1T.bitcast(F32R))
            nc.sync.dma_start(w1t6[D:2 * D, L:128].bitcast(F32R), w1T.bitcast(F32R))

            b12 = mp.tile([128, 1], F32)
            nc.sync.dma_start(b12[0:L, :], b1[:].rearrange("(l one) -> l one", one=1))
            nc.sync.dma_start(b12[L:128, :], b1[:].rearrange("(l one) -> l one", one=1))

            nc.sync.dma_start(pts6[D:2 * D, 1024:HALF].bitcast(F32R),
                              yTd[:, YA + 1024:M].bitcast(F32R))
            nc.gpsimd.dma_start(pts6[0:D, ROWS:HALF].bitcast(F32R),
                                yTd[:, 0:YA].bitcast(F32R))
            # pts6 cols 0:1024 stay unused (chunk 0 reads pts6h instead)

            # W2 scaled by -selu_lambda (folded so h can be alpha*t - r)
            w2t = mp.tile([L, 1], F32)
            nc.sync.dma_start(w2t[:, :], W2[:, :].rearrange("o l -> l o"))
            w2s = mp.tile([L, 1], F32)
            nc.vector.tensor_scalar_mul(w2s[:, :], w2t[:, :], -SELU_L)
            w2stack = mp.tile([128, 2], F32)
            nc.vector.memset(w2stack[:, :], 0.0)
            nc.vector.tensor_copy(w2stack[0:L, 0:1].bitcast(F32R), w2s[:, :])
            nc.vector.tensor_copy(w2stack[L:128, 1:2].bitcast(F32R), w2s[:, :])

            b2b = mp.tile([2, 1], F32)
            nc.sync.dma_start(b2b[0:1, :], b2[:].rearrange("(o one) -> o one", one=1))
            nc.sync.dma_start(b2b[1:2, :], b2[:].rearrange("(o one) -> o one", one=1))

            # early, off-critical-path: clamp/ones rows of the column
            # tensors (rows 0-44 are fully overwritten by DMA loads), packed
            # coords + norms
            nc.gpsimd.memset(ycols[32:46, :], 1.0)
            nc.gpsimd.memset(xcols[32:46, :], CLAMP_EPS)
            # packed coords for |p|^2: y -> [32, 768], x -> [32, 96]
            # (32 partitions so staged chunks are 512B, not 128B)
            # point p*KY+i of y lives at yl[p, 3i:3i+3]
            NPK = 32
            yl = mp.tile([NPK, M * D // NPK], F32)
            nc.gpsimd.dma_start(yl[:, :], y[:, :].flatten().rearrange(
                "(p k) -> p k", p=NPK))
            xl = mp.tile([NPK, ROWS * D // NPK], F32)
            nc.gpsimd.dma_start(xl[:, :], x[:, :].flatten().rearrange(
                "(p k) -> p k", p=NPK))


            # hidden layer: h_pre = blockdiag(W1,W1) @ pts, then per chunk:
            #   e = exp(h+b1) on ACT (f16), and one fused DVE op
            #   hsel = select(h' < 0, alpha*(1-e), -h')  (= -selu(h')/lambda;
            #   lambda folded in W2). z layer + softplus-exp pipelined per
            #   chunk; double-buffered ph PSUM keeps the chain moving.
            CH = 1024
            hsel = mp.tile([128, HALF], F32)
            sp = mp.tile([2, HALF], F32)
            for c0 in range(0, HALF, CH):
                cw = min(CH, HALF - c0)
                ph = mpp.tile([128, CH], F32, tag="ph", bufs=2)
                src = pts6h if c0 == 0 else pts6
                for j in range(0, cw, MMW):
                    jw = min(MMW, cw - j)
                    nc.tensor.matmul(
                        ph[:, j:j + jw],
                        lhsT=w1t6[:, :].bitcast(F32R),
                        rhs=src[:, c0 + j:c0 + j + jw].bitcast(F32R),
                        start=True, stop=True,
                    )
                ec = mt.tile([128, CH], F16, tag="ec")
                nc.scalar.activation(
                    ec[:, 0:cw], ph[:, 0:cw], Act.Exp, bias=b12[:, :])
                nc.vector._custom_dve(
                    selu_op, out=hsel[:, c0:c0 + cw].bitcast(F32R),
                    in0=ph[:, 0:cw], in1=ec[:, 0:cw],
                    s0=b12[:, :], s1=SELU_A,
                )
                pz = mpp.tile([2, CH], F32, tag="pz", bufs=1)
                for j in range(0, cw, MMW):
                    jw = min(MMW, cw - j)
                    nc.tensor.matmul(
                        pz[:, j:j + jw],
                        lhsT=w2stack[:, :].bitcast(F32R),
                        rhs=hsel[:, c0 + j:c0 + j + jw].bitcast(F32R),
                        start=True, stop=True,
                    )
                nc.scalar.activation(
                    sp[:, c0:c0 + cw], pz[:, 0:cw], Act.Exp, bias=b2b[:, :])

            def norms(src, npts, tag):
                k = npts // NPK
                t0 = mp.tile([NPK, k], F32, tag=tag)
                t1 = mp.tile([NPK, k], F32, tag=tag + "b")
                nc.vector.tensor_mul(t0[:, :], src[:, 0::D], src[:, 0::D])
                nc.vector.tensor_mul(t1[:, :], src[:, 1::D], src[:, 1::D])
                nc.vector.tensor_add(t0[:, :], t0[:, :], t1[:, :])
                nc.vector.tensor_mul(t1[:, :], src[:, 2::D], src[:, 2::D])
                nc.vector.tensor_add(t0[:, :], t0[:, :], t1[:, :])
                return t0

            n2yp = norms(yl, M, "nrmy")   # [32, 256], point p*KY+i at [p, i]
            n2xp = norms(xl, ROWS, "nrmx")  # [32, 32]

            nc.scalar.activation(sp[:, :], sp[:, :], Act.Ln, bias=1.0)

            # s in the same packed layout (partition-expand from the sp rows;
            # y-point j: j < YA -> sp[0, ROWS+j], else sp[1, j-YA])
            KY, KX = M // NPK, ROWS // NPK
            PA = YA // KY                 # partitions covered by piece 1
            sxp = mp.tile([NPK, KX], F32)
            nc.sync.dma_start(sxp[:, :], sp[0:1, 0:ROWS])
            sx2p = mp.tile([NPK, KX], F32)
            nc.vector.tensor_mul(sx2p[:, :], sxp[:, :], sxp[:, :])
            syp = mp.tile([NPK, KY], F32)
            nc.sync.dma_start(syp[0:PA, :], sp[0:1, ROWS:HALF])
            nc.sync.dma_start(syp[PA:NPK, :], sp[1:2, :])
            sy2p = mp.tile([NPK, KY], F32)
            nc.vector.tensor_mul(sy2p[:, :], syp[:, :], syp[:, :])

            onesy = mp.tile([NPK, KY], F32)
            nc.vector.memset(onesy[:, :], 1.0)

            # ---- build the 46 matmul columns (fp16 hi/lo split) -------------
            # w~ = sum_p xcol[p](i) * ycol[p](j) = 5*r2*S^2 + CLAMP_EPS
            # p = 3a+b (a<5, b<3); fp16 row triples:
            #   xcols = [Xh; Xh; Xl], ycols = [Yh; Yl; Yh], row 45 clamp*1.
            # Each fp32 column value a is split a = hi + lo with hi = f16(a),
            # lo = f16(a - hi), so XY is recovered to ~2^-22 relative --
            # fp16 matmuls stream 4x faster than fp32 and halve the operand
            # SBUF.
            # x side: f_a in {n2x, 1, x0, x1, x2}, h_b in {sx^2, sx, 1},
            #         coeff ca*cb folded into the x side
            # y side: g_a in {1, n2y, y0, y1, y2}, k_b in {1, sy, sy^2}
            # Products are computed in the packed [128, pts/128] layout, staged
            # to DRAM rows (partition-parallel both ways), then loaded as the
            # [46, pts] matmul operand.
            sfx = nc.next_id()
            yc_stage = nc.dram_tensor(f"yc_stage{sfx}", [30, M], F16)
            xc_stage = nc.dram_tensor(f"xc_stage{sfx}", [30, ROWS], F16)
            ca = [5.0, 5.0, -10.0, -10.0, -10.0]
            cb = [1.0, 2.0, 1.0]
            gy = [onesy, n2yp, yl[:, 0::D], yl[:, 1::D], yl[:, 2::D]]
            ky = [None, syp, sy2p]
            fx = [n2xp, None, xl[:, 0::D], xl[:, 1::D], xl[:, 2::D]]
            hx = [sx2p, sxp, None]
            # x side (small): same big-tile + single copy/split scheme
            prxall32 = mp.tile([NPK, 15 * KX], F32)
            prxall_h = mp.tile([NPK, 15 * KX], F16)
            prxall_l = mp.tile([NPK, 15 * KX], F16)
            for a in range(5):
                for b in range(3):
                    p = 3 * a + b
                    dst = prxall32[:, p * KX:(p + 1) * KX]
                    coeff = ca[a] * cb[b]
                    fa, hb = fx[a], hx[b]
                    if fa is None and hb is None:
                        nc.vector.memset(dst, coeff)
                    elif fa is None:
                        nc.vector.tensor_scalar_mul(dst, hb[:, :], coeff)
                    elif hb is None:
                        nc.vector.tensor_scalar_mul(dst, fa, coeff)
                    else:
                        nc.vector.scalar_tensor_tensor(
                            dst, fa, coeff, hb[:, :],
                            op0=alu.mult, op1=alu.mult)
            nc.scalar.activation(prxall_h[:, :], prxall32[:, :], Act.Copy)
            nc.vector.scalar_tensor_tensor(
                prxall_l[:, :], prxall_h[:, :], -1.0, prxall32[:, :],
                op0=alu.mult, op1=alu.add)
            nc.gpsimd.dma_start(
                xc_stage[0:15, :].rearrange("p (q c) -> q p c", c=KX),
                prxall_h[:, :].rearrange("q (p c) -> q p c", c=KX))
            nc.gpsimd.dma_start(
                xc_stage[15:30, :].rearrange("p (q c) -> q p c", c=KX),
                prxall_l[:, :].rearrange("q (p c) -> q p c", c=KX))

            # x columns; row 45 = (CLAMP_EPS on x) * (1 on y)
            nc.sync.dma_start(xcols[0:15, :], xc_stage[0:15, :])
            nc.gpsimd.dma_start(xcols[15:30, :], xc_stage[0:15, :])
            nc.sync.dma_start(xcols[30:45, :], xc_stage[15:30, :])

            # y-side products into one big f32 tile, then ONE wide ACT copy
            # (hi, f16) + ONE wide DVE stt (lo) for the whole side
            pryall32 = mp.tile([NPK, 15 * KY], F32)
            pryall_h = mp.tile([NPK, 15 * KY], F16)
            pryall_l = mp.tile([NPK, 15 * KY], F16)
            def ysplit(p0, p1):
                c = slice(p0 * KY, p1 * KY)
                nc.scalar.activation(
                    pryall_h[:, c], pryall32[:, c], Act.Copy)
                nc.vector.scalar_tensor_tensor(
                    pryall_l[:, c], pryall_h[:, c], -1.0, pryall32[:, c],
                    op0=alu.mult, op1=alu.add)

            # contiguous-operand products (a<2) go to the otherwise-idle
            # gpsimd engine; strided yl-slice products stay on DVE
            for a in range(5):
                for b in range(3):
                    p = 3 * a + b
                    dst = pryall32[:, p * KY:(p + 1) * KY]
                    ga, kb = gy[a], ky[b]
                    eng = nc.gpsimd if a < 2 else nc.vector
                    if kb is None:
                        eng.tensor_copy(dst, ga)
                    else:
                        eng.tensor_mul(dst, ga, kb[:, :])
                    if p == 7:
                        ysplit(0, 8)
            ysplit(8, 15)
            # stage + load by column quarter so the first matmuls can start
            # before the whole column tensor is assembled
            QC = M // 4
            PQ = QC // KY                  # packed partitions per quarter
            for ci in range(4):
                qs = slice(ci * QC, (ci + 1) * QC)
                ps = slice(ci * PQ, (ci + 1) * PQ)
                # dst row p, col q*KY+c <- src partition q, col p*KY+c
                dst_h = yc_stage[0:15, qs].rearrange(
                    "p (q c) -> q p c", c=KY)
                dst_l = yc_stage[15:30, qs].rearrange(
                    "p (q c) -> q p c", c=KY)
                src_h = pryall_h[ps, :].rearrange("q (p c) -> q p c", c=KY)
                src_l = pryall_l[ps, :].rearrange("q (p c) -> q p c", c=KY)
                nc.sync.dma_start(dst_h, src_h)
                nc.gpsimd.dma_start(dst_l, src_l)
                nc.sync.dma_start(ycols[0:30, qs], yc_stage[:, qs])
                nc.gpsimd.dma_start(ycols[30:45, qs], yc_stage[0:15, qs])



def _main_loop(nc, tc, out, ycols, xcols, tail_op):
    # Per phase (pair of 128-row strips):
    #   [sqrt table]  per strip, per 2048-col group: 4 fp32 K=16 matmuls
    #                 -> PSUM, then ACT sqrt -> strip-wide u tile (fp16)
    #   [exp table]   one strip-wide exp: e3 = exp(-u - ln3)
    #   DVE tail + output DMA per 2048-col group
    # The strip-wide exp reads the whole u tile, so it depends on all 4
    # sqrts of the strip -- ACT cannot interleave exp into the sqrt batch,
    # which would thrash the activation tables.
    with tc.tile_pool(name="main_psum", bufs=2, space="PSUM") as pp, \
         tc.tile_pool(name="upool", bufs=5) as up, \
         tc.tile_pool(name="epool", bufs=3) as ep, \
         tc.tile_pool(name="opool", bufs=3) as op_:
        phase_lens = [4, 2, 1, 1]
        phase_starts = [0, 4, 6, 7]
        for ph0, plen in zip(phase_starts, phase_lens):
            strips = range(ph0, ph0 + plen)
            utiles = {}
            for s in strips:
                lhs = xcols[:, s * 128:(s + 1) * 128]
                u = up.tile([128, M], F16, tag="u")
                utiles[s] = u
                for g in range(N_GRP):
                    pw = pp.tile([128, GRP], F32, tag="pw")
                    for j in range(0, GRP, MMW):
                        nc.tensor.matmul(
                            pw[:, j:j + MMW],
                            lhsT=lhs,
                            rhs=ycols[:, g * GRP + j:g * GRP + j + MMW],
                            start=True, stop=True,
                        )
                    nc.scalar.activation(
                        u[:, g * GRP:(g + 1) * GRP], pw[:, :], Act.Sqrt)
            last_phase = ph0 + plen >= N_STRIPS
            etiles = {}
            for s in strips:
                e3 = ep.tile([128, M], F16, tag="e3")
                etiles[s] = e3
                if last_phase and s == strips[-1]:
                    # split the final exp so the tail DVE ops + out DMAs
                    # pipeline into the drain instead of waiting 8192-wide
                    for g in range(N_GRP):
                        sl = slice(g * GRP, (g + 1) * GRP)
                        nc.scalar.activation(
                            e3[:, sl], utiles[s][:, sl], Act.Exp,
                            bias=-LN3, scale=-1.0)
                else:
                    nc.scalar.activation(
                        e3[:, :], utiles[s][:, :], Act.Exp,
                        bias=-LN3, scale=-1.0)
            for s in strips:
                for g in range(N_GRP):
                    sl = slice(g * GRP, (g + 1) * GRP)
                    o = op_.tile([128, GRP], F16, tag="o")
                    nc.vector._custom_dve(
                        tail_op, out=o[:, :], in0=utiles[s][:, sl],
                        in1=etiles[s][:, sl], s0=1.5, s1=0.75,
                    )
                    nc.sync.dma_start(
                        out[s * 128:(s + 1) * 128, g * GRP:(g + 1) * GRP],
                        o[:, :],
                    )


_NC_CACHE = None


def kernel(**inputs):
    global _NC_CACHE
    if _NC_CACHE is None:
        _NC_CACHE = build()
    nc = _NC_CACHE
    x = np.ascontiguousarray(np.asarray(inputs["x"], dtype=np.float32))
    yf = np.ascontiguousarray(np.asarray(inputs["y"], dtype=np.float32))
    base = {
        "y": yf,
        "yT": np.ascontiguousarray(yf.T),
        "W1": np.ascontiguousarray(np.asarray(inputs["W1"], dtype=np.float32)),
        "b1": np.ascontiguousarray(np.asarray(inputs["b1"], dtype=np.float32)),
        "W2": np.ascontiguousarray(np.asarray(inputs["W2"], dtype=np.float32)),
        "b2": np.ascontiguousarray(np.asarray(inputs["b2"], dtype=np.float32)),
    }
    in_maps = [
        {"x": x[c * ROWS:(c + 1) * ROWS],
         "xT": np.ascontiguousarray(x[c * ROWS:(c + 1) * ROWS].T), **base}
        for c in range(N_CORES)
    ]
    res = run_bass_kernel_spmd(nc, in_maps, core_ids=list(range(N_CORES)))
    return np.concatenate(
        [res.results[c]["out"] for c in range(N_CORES)], axis=0
    ).astype(np.float32)

